# revision 15
# baseline (speedup 1.0000x reference)
"""Trainium2 kernel for nn_CaptionGenerator.

Strategy: the recurrence (LSTM + region attention) is decoupled from the
vocab dimension V=12000 because the only full-V quantity feeding back into
the recurrence is wt.sum(axis=1) = h @ sum_v(P_wh) + sum_v(P_w), which
collapses to a dot with a precomputed [H] vector.  The heavy [B,T,V]
predictions tensor is then one big matmul, vocab-sharded over the 8
NeuronCores with zero collectives:

    predsT[v, (b,t)] = sum_k A[k, v] * M[k, (b,t)]

with K = 770 rows:
    A = [ P_whT (512) ; embT (256) ; P_w (1) ; ones (1) ]   (per-core V slice)
    M = [ mask*h_t (512) ; mask*U (256) ; mask (1) ; mask*(rvs+c_b) (1) ]

P_whT (the [V,H] vocab projection, transposed) is computed on-device from
the core's emb_W slice.  sumr_wr = obj_sum @ P_wr.T is folded through the
embT rows via U = obj_sum @ wr_W, avoiding materializing P_wr entirely.
"""

import numpy as np

B, T, V, E, H, D, R = 64, 21, 12000, 256, 512, 1024, 36
NC = 8
VC = V // NC          # 1500 vocab rows per core
TD = T - 1            # 20 decode steps
BT = B * TD           # 1280
KE = H + E + 2        # 770 contraction rows

_cache = {}
last_results = None
last_wall_ns = None


def _build_program():
    import concourse.bass as bass
    import concourse.mybir as mybir
    from concourse.tile import TileContext
    from concourse.tile_rust import add_dep_helper

    f32 = mybir.dt.float32
    f32r = mybir.dt.float32r
    X = VC + BT + VC + BT                                 # 5560 blob cols

    nc = bass.Bass()
    blob_d = nc.declare_dram_parameter("blob", [2 * 128, X], f32r, isOutput=False)
    out_d = nc.declare_dram_parameter("predsT", [12 * 128, BT], f32, isOutput=True)

    NCH2 = [(0, 512), (512, 512), (1024, 256)]
    NVT = (VC + 127) // 128

    with TileContext(nc) as tc:
        with (
            tc.tile_pool(name="const", bufs=1) as const,
            tc.tile_pool(name="work", bufs=1) as work,
            tc.tile_pool(name="psum", bufs=6, space="PSUM") as psum,
        ):
            blob_t = const.tile([128, 2, X], f32r, tag="blob")
            blob_dma = nc.gpsimd.dma_start(
                out=blob_t[:], in_=blob_d.rearrange("(a p) n -> p a n", p=128))
            big_ot = work.tile([128, 12, BT], f32, tag="bigout")
            last_mm = None
            last_cp = None

            for vt in range(NVT):
                v0 = vt * 128
                mw = min(128, VC - v0)
                for (n0, nw) in NCH2:
                    ps_t = psum.tile([128, 512], f32, tag="ps")
                    ps = ps_t[:, :nw]
                    for j in range(2):
                        nc.tensor.matmul(
                            ps[:mw, :],
                            blob_t[:, j, v0:v0 + mw],
                            blob_t[:, j, VC + n0:VC + n0 + nw],
                            start=(j == 0), stop=False,
                        )
                    last_mm = nc.tensor.matmul(
                        ps[:mw, :],
                        blob_t[0:2, 0, VC + BT + v0:VC + BT + v0 + mw],
                        blob_t[0:2, 0, VC + BT + VC + n0:VC + BT + VC + n0 + nw],
                        start=False, stop=True,
                    )
                    last_cp = nc.vector.tensor_copy(
                        big_ot[:mw, vt, n0:n0 + nw], ps[:mw, :])
            out_dma = nc.gpsimd.dma_start(
                out=out_d.rearrange("(a p) n -> p a n", p=128), in_=big_ot[:])
            for dep in (blob_dma, last_mm, last_cp, out_dma):
                nop = nc.sync.nop()
                add_dep_helper(
                    nop.ins, dep.ins, reason="pre-observe for tail drain")
    return nc


def _sigmoid(x):
    return 1.0 / (1.0 + np.exp(-x))


def _softmax(x, axis):
    m = np.max(x, axis=axis, keepdims=True)
    e = np.exp(x - m)
    return e / np.sum(e, axis=axis, keepdims=True)


def kernel(h0, object_proposals, captions, caption_lengths,
           emb_W, wh_W, wh_b, wr_W, wr_b, rh_W, rh_b, w_W, w_b, r_W, r_b,
           lstm_Wih, lstm_Whh, lstm_bih, lstm_bhh):
    global last_results, last_wall_ns
    import time

    h0 = np.asarray(h0, np.float32)
    object_proposals = np.asarray(object_proposals, np.float32)
    captions_np = np.asarray(captions)
    caption_lengths_np = np.asarray(caption_lengths)
    emb_W = np.asarray(emb_W, np.float32)
    wh_W = np.asarray(wh_W, np.float32); wh_b = np.asarray(wh_b, np.float32)
    wr_W = np.asarray(wr_W, np.float32); wr_b = np.asarray(wr_b, np.float32)
    rh_W = np.asarray(rh_W, np.float32); rh_b = np.asarray(rh_b, np.float32)
    w_W = np.asarray(w_W, np.float32); w_b = np.asarray(w_b, np.float32)
    r_W = np.asarray(r_W, np.float32); r_b = np.asarray(r_b, np.float32)
    Wih = np.asarray(lstm_Wih, np.float32); Whh = np.asarray(lstm_Whh, np.float32)
    bih = np.asarray(lstm_bih, np.float32); bhh = np.asarray(lstm_bhh, np.float32)

    # ---- shard-prep / index ops (host) --------------------------------
    lens = caption_lengths_np[:, 0]
    sort_ind = np.argsort(-lens, kind="stable")
    lens_s = lens[sort_ind]
    dec = (lens_s - 1).astype(caption_lengths_np.dtype)
    h0s = h0[sort_ind]
    objs = object_proposals[sort_ind]
    caps = captions_np[sort_ind]
    emb = emb_W[caps]                                   # [B,T,E]

    # time-invariant small projections
    rh_all = objs @ rh_W.T + rh_b                       # [B,R,H]
    r_lin = objs @ r_W[0] + r_b[0]                      # [B,R]
    obj_sum = objs.sum(axis=1)                          # [B,D]
    colsum = emb_W.sum(axis=0)                          # [E]
    s_wh = colsum @ wh_W.T + V * wh_b                   # [H]
    s_wr = colsum @ wr_W.T + V * wr_b                   # [D]
    s_w = float(colsum @ w_W[0] + V * w_b[0])
    sumv_wr = objs @ s_wr                               # [B,R]
    U = obj_sum @ wr_W                                  # [B,E]
    c_b = obj_sum @ wr_b                                # [B]
    P_w = emb_W @ w_W[0] + w_b[0]                       # [V]

    # ---- tiny sequential recurrence (B=64, 20 steps) ------------------
    h = h0s.copy(); c = h0s.copy()
    rf = np.zeros((B, D), np.float32)
    G = np.zeros((H, BT), np.float32)
    maskU = np.zeros((E, BT), np.float32)
    mex = np.zeros((2, BT), np.float32)
    attention = np.zeros((B, TD, R), np.float32)
    for t in range(TD):
        mask = dec > t                                  # [B]
        mf = mask.astype(np.float32)
        x = np.concatenate([emb[:, t, :], rf], axis=1)
        gates = x @ Wih.T + bih + h @ Whh.T + bhh
        i_, f_, g_, o_ = np.split(gates, 4, axis=1)
        i_ = _sigmoid(i_); f_ = _sigmoid(f_); o_ = _sigmoid(o_)
        g_ = np.tanh(g_)
        c_new = f_ * c + i_ * g_
        h_new = o_ * np.tanh(c_new)
        h = np.where(mask[:, None], h_new, h)
        c = np.where(mask[:, None], c_new, c)
        rvec = np.einsum("brh,bh->br", rh_all, h) + r_lin
        wtsum = h @ s_wh + s_w
        logits = wtsum[:, None] + sumv_wr + rvec
        ra = _softmax(logits, axis=1)
        rf_new = np.einsum("br,brd->bd", ra, objs)
        rf = np.where(mask[:, None], rf_new, rf)
        attention[:, t, :] = ra * mf[:, None]
        G[:, t::TD] = (h * mf[:, None]).T
        maskU[:, t::TD] = (U * mf[:, None]).T
        mex[0, t::TD] = mf
        mex[1, t::TD] = mf * (rvec.sum(axis=1) + c_b)

    M = np.ascontiguousarray(
        np.concatenate([G, maskU, mex], axis=0), np.float32)   # [770, BT]

    # ---- device: vocab-sharded big matmul over 8 cores ----------------
    from concourse.bass_utils import run_bass_kernel_spmd

    if "nc" not in _cache:
        _cache["nc"] = _build_program()
    ncprog = _cache["nc"]

    GW = wh_W.T.astype(np.float32) @ G                   # [E, BT]
    whbG = wh_b @ G                                      # [BT]
    M2 = GW + maskU                                      # [E, BT]
    embT = np.ascontiguousarray(emb_W.T, np.float32)     # [E, V]
    X = VC + BT + VC + BT
    in_maps = []
    for k in range(NC):
        sl = slice(k * VC, (k + 1) * VC)
        blob = np.zeros((256, X), np.float32)
        blob[:, 0:VC] = embT[:, sl]
        blob[:, VC:VC + BT] = M2
        blob[0, VC + BT:VC + BT + VC] = P_w[sl]
        blob[1, VC + BT:VC + BT + VC] = 1.0
        blob[0, VC + BT + VC:] = mex[0]
        blob[1, VC + BT + VC:] = mex[1] + whbG
        in_maps.append({"blob": blob})
    t0 = time.perf_counter_ns()
    res = run_bass_kernel_spmd(ncprog, in_maps, list(range(NC)))
    last_wall_ns = time.perf_counter_ns() - t0
    last_results = res

    predsT = np.concatenate(
        [np.asarray(res.results[k]["predsT"])[:VC] for k in range(NC)], axis=0)
    predictions = np.ascontiguousarray(predsT.T).reshape(B, TD, V)

    caps_out = caps[:, 1:].astype(captions_np.dtype)
    return (predictions, attention, caps_out, dec,
            sort_ind.astype(np.int32))


# revision 16
# speedup vs baseline: 1.3552x; 1.3552x over previous
"""Trainium2 kernel for nn_CaptionGenerator.

Strategy: the recurrence (LSTM + region attention) is decoupled from the
vocab dimension V=12000 because the only full-V quantity feeding back into
the recurrence is wt.sum(axis=1) = h @ sum_v(P_wh) + sum_v(P_w), which
collapses to a dot with a precomputed [H] vector.  The heavy [B,T,V]
predictions tensor is then one big matmul, vocab-sharded over the 8
NeuronCores with zero collectives:

    predsT[v, (b,t)] = sum_k A[k, v] * M[k, (b,t)]

with K = 770 rows:
    A = [ P_whT (512) ; embT (256) ; P_w (1) ; ones (1) ]   (per-core V slice)
    M = [ mask*h_t (512) ; mask*U (256) ; mask (1) ; mask*(rvs+c_b) (1) ]

P_whT (the [V,H] vocab projection, transposed) is computed on-device from
the core's emb_W slice.  sumr_wr = obj_sum @ P_wr.T is folded through the
embT rows via U = obj_sum @ wr_W, avoiding materializing P_wr entirely.
"""

import numpy as np

B, T, V, E, H, D, R = 64, 21, 12000, 256, 512, 1024, 36
NC = 8
VC = V // NC          # 1500 vocab rows per core
TD = T - 1            # 20 decode steps
BT = B * TD           # 1280
KE = H + E + 2        # 770 contraction rows

_cache = {}
last_results = None
last_wall_ns = None


def _build_program():
    import concourse.bass as bass
    import concourse.mybir as mybir
    from concourse.tile import TileContext
    from concourse.tile_rust import add_dep_helper

    f32 = mybir.dt.float32
    f32r = mybir.dt.float32r
    X = VC + BT + VC + BT                                 # 5560 blob cols

    nc = bass.Bass()
    blob0_d = nc.declare_dram_parameter("blob0", [128, X], f32r, isOutput=False)
    blob1_d = nc.declare_dram_parameter("blob1", [128, VC + BT], f32r, isOutput=False)
    out_d = nc.declare_dram_parameter("predsT", [12 * 128, BT], f32, isOutput=True)

    NCH2 = [(0, 512), (512, 512), (1024, 256)]
    NVT = (VC + 127) // 128

    with TileContext(nc) as tc:
        with (
            tc.tile_pool(name="const", bufs=1) as const,
            tc.tile_pool(name="work", bufs=1) as work,
            tc.tile_pool(name="psum", bufs=6, space="PSUM") as psum,
        ):
            blob0_t = const.tile([128, X], f32r, tag="blob0")
            blob_dma0 = nc.gpsimd.dma_start(out=blob0_t[:], in_=blob0_d[:])
            blob1_t = const.tile([128, VC + BT], f32r, tag="blob1")
            blob_dma1 = nc.gpsimd.dma_start(out=blob1_t[:], in_=blob1_d[:])
            blobs = [blob0_t, blob1_t]
            big0 = work.tile([128, 6, BT], f32, tag="big0")
            big1 = work.tile([128, 6, BT], f32, tag="big1")
            last_mm = None
            last_cp0 = None
            last_cp1 = None

            for vt in range(NVT):
                v0 = vt * 128
                mw = min(128, VC - v0)
                for (n0, nw) in NCH2:
                    ps_t = psum.tile([128, 512], f32, tag="ps")
                    ps = ps_t[:, :nw]
                    for j in range(2):
                        nc.tensor.matmul(
                            ps[:mw, :],
                            blobs[j][:, v0:v0 + mw],
                            blobs[j][:, VC + n0:VC + n0 + nw],
                            start=(j == 0), stop=False,
                        )
                    last_mm = nc.tensor.matmul(
                        ps[:mw, :],
                        blob0_t[0:2, VC + BT + v0:VC + BT + v0 + mw],
                        blob0_t[0:2, VC + BT + VC + n0:VC + BT + VC + n0 + nw],
                        start=False, stop=True,
                    )
                    if vt < 6:
                        last_cp0 = nc.vector.tensor_copy(
                            big0[:mw, vt, n0:n0 + nw], ps[:mw, :])
                    else:
                        last_cp1 = nc.vector.tensor_copy(
                            big1[:mw, vt - 6, n0:n0 + nw], ps[:mw, :])
            out_dma0 = nc.gpsimd.dma_start(
                out=out_d[0:768, :].rearrange("(a p) n -> p a n", p=128),
                in_=big0[:])
            out_dma1 = nc.gpsimd.dma_start(
                out=out_d[768:1536, :].rearrange("(a p) n -> p a n", p=128),
                in_=big1[:])
            for dep in (blob_dma0, blob_dma1, last_mm, last_cp0, last_cp1,
                        out_dma0, out_dma1):
                nop = nc.sync.nop()
                add_dep_helper(
                    nop.ins, dep.ins, reason="pre-observe for tail drain")
    return nc


def _sigmoid(x):
    return 1.0 / (1.0 + np.exp(-x))


def _softmax(x, axis):
    m = np.max(x, axis=axis, keepdims=True)
    e = np.exp(x - m)
    return e / np.sum(e, axis=axis, keepdims=True)


def kernel(h0, object_proposals, captions, caption_lengths,
           emb_W, wh_W, wh_b, wr_W, wr_b, rh_W, rh_b, w_W, w_b, r_W, r_b,
           lstm_Wih, lstm_Whh, lstm_bih, lstm_bhh):
    global last_results, last_wall_ns
    import time

    h0 = np.asarray(h0, np.float32)
    object_proposals = np.asarray(object_proposals, np.float32)
    captions_np = np.asarray(captions)
    caption_lengths_np = np.asarray(caption_lengths)
    emb_W = np.asarray(emb_W, np.float32)
    wh_W = np.asarray(wh_W, np.float32); wh_b = np.asarray(wh_b, np.float32)
    wr_W = np.asarray(wr_W, np.float32); wr_b = np.asarray(wr_b, np.float32)
    rh_W = np.asarray(rh_W, np.float32); rh_b = np.asarray(rh_b, np.float32)
    w_W = np.asarray(w_W, np.float32); w_b = np.asarray(w_b, np.float32)
    r_W = np.asarray(r_W, np.float32); r_b = np.asarray(r_b, np.float32)
    Wih = np.asarray(lstm_Wih, np.float32); Whh = np.asarray(lstm_Whh, np.float32)
    bih = np.asarray(lstm_bih, np.float32); bhh = np.asarray(lstm_bhh, np.float32)

    # ---- shard-prep / index ops (host) --------------------------------
    lens = caption_lengths_np[:, 0]
    sort_ind = np.argsort(-lens, kind="stable")
    lens_s = lens[sort_ind]
    dec = (lens_s - 1).astype(caption_lengths_np.dtype)
    h0s = h0[sort_ind]
    objs = object_proposals[sort_ind]
    caps = captions_np[sort_ind]
    emb = emb_W[caps]                                   # [B,T,E]

    # time-invariant small projections
    rh_all = objs @ rh_W.T + rh_b                       # [B,R,H]
    r_lin = objs @ r_W[0] + r_b[0]                      # [B,R]
    obj_sum = objs.sum(axis=1)                          # [B,D]
    colsum = emb_W.sum(axis=0)                          # [E]
    s_wh = colsum @ wh_W.T + V * wh_b                   # [H]
    s_wr = colsum @ wr_W.T + V * wr_b                   # [D]
    s_w = float(colsum @ w_W[0] + V * w_b[0])
    sumv_wr = objs @ s_wr                               # [B,R]
    U = obj_sum @ wr_W                                  # [B,E]
    c_b = obj_sum @ wr_b                                # [B]
    P_w = emb_W @ w_W[0] + w_b[0]                       # [V]

    # ---- tiny sequential recurrence (B=64, 20 steps) ------------------
    h = h0s.copy(); c = h0s.copy()
    rf = np.zeros((B, D), np.float32)
    G = np.zeros((H, BT), np.float32)
    maskU = np.zeros((E, BT), np.float32)
    mex = np.zeros((2, BT), np.float32)
    attention = np.zeros((B, TD, R), np.float32)
    for t in range(TD):
        mask = dec > t                                  # [B]
        mf = mask.astype(np.float32)
        x = np.concatenate([emb[:, t, :], rf], axis=1)
        gates = x @ Wih.T + bih + h @ Whh.T + bhh
        i_, f_, g_, o_ = np.split(gates, 4, axis=1)
        i_ = _sigmoid(i_); f_ = _sigmoid(f_); o_ = _sigmoid(o_)
        g_ = np.tanh(g_)
        c_new = f_ * c + i_ * g_
        h_new = o_ * np.tanh(c_new)
        h = np.where(mask[:, None], h_new, h)
        c = np.where(mask[:, None], c_new, c)
        rvec = np.einsum("brh,bh->br", rh_all, h) + r_lin
        wtsum = h @ s_wh + s_w
        logits = wtsum[:, None] + sumv_wr + rvec
        ra = _softmax(logits, axis=1)
        rf_new = np.einsum("br,brd->bd", ra, objs)
        rf = np.where(mask[:, None], rf_new, rf)
        attention[:, t, :] = ra * mf[:, None]
        G[:, t::TD] = (h * mf[:, None]).T
        maskU[:, t::TD] = (U * mf[:, None]).T
        mex[0, t::TD] = mf
        mex[1, t::TD] = mf * (rvec.sum(axis=1) + c_b)

    M = np.ascontiguousarray(
        np.concatenate([G, maskU, mex], axis=0), np.float32)   # [770, BT]

    # ---- device: vocab-sharded big matmul over 8 cores ----------------
    from concourse.bass_utils import run_bass_kernel_spmd

    if "nc" not in _cache:
        _cache["nc"] = _build_program()
    ncprog = _cache["nc"]

    GW = wh_W.T.astype(np.float32) @ G                   # [E, BT]
    whbG = wh_b @ G                                      # [BT]
    M2 = GW + maskU                                      # [E, BT]
    embT = np.ascontiguousarray(emb_W.T, np.float32)     # [E, V]
    X = VC + BT + VC + BT
    in_maps = []
    for k in range(NC):
        sl = slice(k * VC, (k + 1) * VC)
        blob = np.zeros((256, X), np.float32)
        blob[:, 0:VC] = embT[:, sl]
        blob[:, VC:VC + BT] = M2
        blob[0, VC + BT:VC + BT + VC] = P_w[sl]
        blob[1, VC + BT:VC + BT + VC] = 1.0
        blob[0, VC + BT + VC:] = mex[0]
        blob[1, VC + BT + VC:] = mex[1] + whbG
        in_maps.append({"blob0": np.ascontiguousarray(blob[:128]),
                        "blob1": np.ascontiguousarray(blob[128:, :VC + BT])})
    t0 = time.perf_counter_ns()
    res = run_bass_kernel_spmd(ncprog, in_maps, list(range(NC)))
    last_wall_ns = time.perf_counter_ns() - t0
    last_results = res

    predsT = np.concatenate(
        [np.asarray(res.results[k]["predsT"])[:VC] for k in range(NC)], axis=0)
    predictions = np.ascontiguousarray(predsT.T).reshape(B, TD, V)

    caps_out = caps[:, 1:].astype(captions_np.dtype)
    return (predictions, attention, caps_out, dec,
            sort_ind.astype(np.int32))
